# revision 32
# baseline (speedup 1.0000x reference)
"""NetVLAD (vq_codebook) Trainium2 Bass kernel, 8-way spatially sharded. v3.

Math (same identity as v1/v2):
  xn = x / ||x||_C per location; logits = conv_w @ xn; soft = softmax_K
  fold(unfold(soft) * top2keep) == soft * cnt, cnt = 3x3 box-sum of the
  per-cluster top-2 indicator (border wrap artifacts killed by the
  (min-dist-to-border)^4 mask). vlad = sa2 @ xn.T - rowsum(sa2) * centroids.

v3 strategy (all compute in [L-partition, K-free] layout, no transposes):
  - x is L2-normalized on the host and streamed quantized in BOTH layouts:
    fp8e4m3 [C,L]-tiled for the logits lhsT, bf16 [L,C+1] for the VLAD
    moving operand (+ a ones column that yields rowsum(sa2) for free).
  - logits computed directly into [l-tile, K] PSUM banks, 4 tiles/bank;
    one batched exp per bank (no scale, no accumulator read).
  - softmax sum + top-2 keep on DVE in bf16 chunks; cnt = 3x3 box-sum on
    the PE as 5 banded 128x128 0/1 matmuls per tile, d-outer waves.
  - per-core [K, C+1] partials reduced on host (0.03% of FLOPs).
  - xvb stream chunks are gated behind scalar-engine progress via
    WAR deps (a scalar touch reads the chunk region before the DMA
    writes it) so the Tile scheduler cannot front-run the x load.

Sharding: H=192 rows split 8 ways (24 rows/core + 1 halo row each side).
"""
import os
import sys

sys.path.insert(0, "/opt/trn_rl_repo")
os.environ.setdefault("MYCRO_LOCAL_CACHE", "1")

import numpy as np

C, H, W, K = 512, 192, 192, 64
M = 8                       # cores
RPC = H // M                # 24 rows per core
Ls = (RPC + 2) * W          # 4992 slab locations (incl. 1 halo row each side)
NT = Ls // 128              # 39 l-tiles
CT = C // 128               # 4 c-tiles
C1 = C + 1                  # x columns + ones column
XTG = [6, 9, 11, 13]        # xtb DMA chunk sizes (tiles), staggered completion
TPB = [(0, 10), (10, 20), (20, 30), (30, 39)]   # top2 chunks
CNW = [(1, 8), (8, 18), (18, 28), (28, 38)]     # cnt waves == vlad groups
EB = 4                      # exp batch: tiles per PSUM bank
GATES = {0: 0, 2: 1, 4: 2, 6: 3}  # exp-batch index -> xvb chunk after it

TRACE = False               # set by test.py for profiling runs
_CACHE = {}


def _build_nc():
    import concourse.bass as bass
    import concourse.tile as tile
    from concourse import mybir

    f32 = mybir.dt.float32
    bf16 = mybir.dt.bfloat16
    fp8 = mybir.dt.float8e4
    AF = mybir.ActivationFunctionType
    OP = mybir.AluOpType
    AX = mybir.AxisListType

    NV = CNW[-1][1] - CNW[0][0]        # 37 tiles carried in xvb (skip 0, 38)
    V0 = CNW[0][0]

    nc = bass.Bass()
    xtb = nc.dram_tensor("xtb", [128, NT * CT * 128], fp8, kind="ExternalInput")
    xvb = nc.dram_tensor("xvb", [128, NV * C1], bf16, kind="ExternalInput")
    cwb = nc.dram_tensor("cwb", [128, CT * K], fp8, kind="ExternalInput")
    shb = nc.dram_tensor("shb", [128, 5 * 128], bf16, kind="ExternalInput")
    cst = nc.dram_tensor("cst", [128, NT], f32, kind="ExternalInput")
    y = nc.dram_tensor("y", [K, C1], f32, kind="ExternalOutput")

    xtg = np.cumsum([0] + XTG)

    with tile.TileContext(nc) as tc:
        with tc.tile_pool(name="big", bufs=1) as big:
            xtb_sb = big.tile([128, NT * CT * 128], fp8, tag="xtb")
            xvb_sb = big.tile([128, NV * C1], bf16, tag="xvb")
            cwb_sb = big.tile([128, CT * K], fp8, tag="cwb")
            shb_sb = big.tile([128, 5 * 128], bf16, tag="shb")
            sc0 = big.tile([128, NT], f32, tag="sc0")
            expb = big.tile([128, NT * K], bf16, tag="expb")
            keep = big.tile([128, NT * K], bf16, tag="keep")
            seb = big.tile([128, NT * K], bf16, tag="seb")
            w2b = big.tile([128, NT * K], bf16, tag="w2b")
            sume = big.tile([128, NT], f32, tag="sume")
            isum = big.tile([128, NT], f32, tag="isum")
            scc = big.tile([128, NT], f32, tag="scc")
            m8 = big.tile([128, NT * 8], bf16, tag="m8")
            vl_sb = big.tile([K, C1], f32, tag="vl")
            scr = big.tile([128, 4], f32, tag="scr")

            # cwb (needed by the first warmup + phase 1) goes first on the
            # sync DGE; the other constants ride the scalar HWDGE so the
            # xtb stream descriptors start generating immediately after
            nc.sync.dma_start(cwb_sb[:], cwb[:])
            nc.scalar.dma_start(shb_sb[:], shb[:])
            nc.scalar.dma_start(sc0[:], cst[:])
            for g in range(len(XTG)):
                a, b = int(xtg[g]) * CT * 128, int(xtg[g + 1]) * CT * 128
                nc.sync.dma_start(xtb_sb[:, a:b], xtb[:, a:b])

            # single-wait touch absorbing the cst DMA for later DVE/scalar use
            nc.scalar.copy(scr[:, 0:1], sc0[:, 0:1])
            # init the xvb gate columns so the gate touches read defined data
            for (va, _vb) in CNW:
                a = (va - V0) * C1
                nc.vector.memset(xvb_sb[:, a:a + 1], 0.0)

            with tc.tile_pool(name="pp", bufs=1, space="PSUM") as pp:
                # all PSUM tiles are whole-bank (2048B) multiples so every
                # allocation is bank-aligned and matmuls never cross banks:
                # pcn 2 + pv0 1 + pv1 1 + plg 3 + dum 1 = 8 banks
                pcn = pp.tile([128, 2 * C], f32, tag="pcn", bufs=1)
                pv0 = pp.tile([K, C], f32, tag="pv0", bufs=1)
                pv1 = pp.tile([K, C], f32, tag="pv1", bufs=1)
                plg0 = pp.tile([128, C], f32, tag="plg", bufs=3,
                               name="plg0")  # reserve bank-aligned slots

                # warm-up burst: absorbs cwb/shb DMA waits, spins up the HAM
                dummy = pp.tile([128, K], f32, tag="dum", bufs=1)
                nc.tensor.matmul(dummy[0:64, 0:64], lhsT=cwb_sb[:, 0:64],
                                 rhs=cwb_sb[:, 0:64], start=True, stop=True)
                for _ in range(6):
                    dummy = pp.tile([128, K], f32, tag="dum", bufs=1)
                    nc.tensor.matmul(dummy[:, 0:64], lhsT=shb_sb[:, 0:128],
                                     rhs=shb_sb[:, 0:64], start=True, stop=True)

                # ---- phase 1: logits into [l-tile, K] PSUM, 4 tiles/bank;
                # batched exp per bank (x pre-normalized, so no scale)
                nb = (NT + EB - 1) // EB
                gate_cols = {}
                plogs = {}
                cg = 0                      # xtb chunk cursor

                DR = mybir.MatmulPerfMode.DoubleRow
                for bi in range(nb):
                    t0, t1 = bi * EB, min(NT, (bi + 1) * EB)
                    plg = pp.tile([128, C], f32, tag="plg", bufs=3)
                    plogs[bi] = plg
                    for t in range(t0, t1):
                        if cg < len(XTG) and t == int(xtg[cg]):
                            # dummy matmul absorbs this xtb chunk's DMA wait
                            dummy = pp.tile([128, K], f32, tag="dum", bufs=1)
                            base = t * CT * 128
                            nc.tensor.matmul(
                                dummy[:, 0:64],
                                lhsT=xtb_sb[:, base:base + 128],
                                rhs=xtb_sb[:, base:base + 64],
                                start=True, stop=True)
                            cg += 1
                        # fp8 DoubleRow: 256-deep contraction per matmul,
                        # c-halves stacked in the AP middle dim
                        for h in range(2):
                            lv = xtb_sb[:, (t * CT + 2 * h) * 128:
                                        (t * CT + 2 * h + 2) * 128]
                            rv = cwb_sb[:, 2 * h * K:(2 * h + 2) * K]
                            nc.tensor.matmul(
                                plg[:, (t - t0) * K:(t - t0 + 1) * K],
                                lhsT=lv.rearrange("p (two f) -> p two f",
                                                  two=2),
                                rhs=rv.rearrange("p (two f) -> p two f",
                                                 two=2),
                                start=(h == 0), stop=(h == 1),
                                perf_mode=DR,
                            )
                    nc.scalar.activation(
                        expb[:, t0 * K:t1 * K], plg[:, 0:(t1 - t0) * K],
                        AF.Exp)
                    if bi in GATES:
                        # gate: touch-read the xvb chunk region, then issue
                        # its DMA; the WAR dep paces the stream off scalar
                        # progress (the Tile scheduler keeps DMA after touch)
                        gi = GATES[bi]
                        va, vb = CNW[gi]
                        a = (va - V0) * C1
                        b = (vb - V0) * C1
                        nc.scalar.copy(scr[:, 2:3], xvb_sb[:, a:a + 1])
                        nc.sync.dma_start(xvb_sb[:, a:b], xvb[:, a:b])

                # ---- phases 2-4, software-pipelined: top2 chunk ci, then
                # wave ci-1's [cnt matmuls -> scalar drain -> w2 -> VLAD]
                # (wave i only needs keep from chunk i, so trailing by one
                # chunk keeps every engine busy without stalls)

                def top2_keep(ci):
                    ta, tb = TPB[ci]
                    n = tb - ta
                    e3 = expb[:, ta * K:tb * K].rearrange(
                        "p (t k) -> p t k", k=K)
                    k3 = keep[:, ta * K:tb * K].rearrange(
                        "p (t k) -> p t k", k=K)
                    # per-tile top-8 (Max8); m2 = column 1
                    for t in range(ta, tb):
                        nc.vector.max(m8[:, t * 8:(t + 1) * 8],
                                      expb[:, t * K:(t + 1) * K])
                    m83 = m8[:, ta * 8:tb * 8].rearrange(
                        "p (t e) -> p t e", e=8)
                    m2c = m83[:, :, 1:2].broadcast_to([128, n, K])
                    nc.vector.tensor_tensor(k3, e3, m2c, op=OP.is_ge)

                def top2_tail(ci):
                    ta, tb = TPB[ci]
                    n = tb - ta
                    e3 = expb[:, ta * K:tb * K].rearrange(
                        "p (t k) -> p t k", k=K)
                    s3 = seb[:, ta * K:tb * K].rearrange(
                        "p (t k) -> p t k", k=K)
                    scc_c = scc[:, ta:tb][:, :, None].broadcast_to([128, n, K])
                    nc.vector.tensor_reduce(
                        sume[:, ta:tb], e3, axis=AX.X, op=OP.add)
                    nc.vector.reciprocal(isum[:, ta:tb], sume[:, ta:tb])
                    nc.vector.tensor_mul(
                        scc[:, ta:tb], sc0[:, ta:tb], isum[:, ta:tb])
                    nc.vector.tensor_tensor(s3, e3, scc_c, op=OP.mult)

                def wave(wi):
                    wa, wb = CNW[wi]
                    # dummy absorbs this wave's keep (DVE) wait
                    dummy = pp.tile([128, K], f32, tag="dum", bufs=1)
                    nc.tensor.matmul(
                        dummy[:, 0:K],
                        lhsT=shb_sb[:, 0:128],
                        rhs=keep[:, wa * K:(wa + 1) * K],
                        start=True, stop=True)
                    # d-outer banded matmuls: one LDWEIGHTS per shift matrix.
                    # start/stop once per 2KB PSUM zero region (8 tiles/bank):
                    # start=True lazily zeroes the bank; first-touch writes of
                    # other tiles store, later touches accumulate.
                    pairs = [(d, t) for d in range(-2, 3)
                             for t in range(wa, wb) if 0 <= t + d < NT]
                    bank = lambda t: (t - wa) // 8
                    fidx = {}
                    lidx = {}
                    for i, (d, t) in enumerate(pairs):
                        fidx.setdefault(bank(t), i)
                        lidx[bank(t)] = i
                    for i, (d, t) in enumerate(pairs):
                        nc.tensor.matmul(
                            pcn[:, (t - wa) * K:(t - wa + 1) * K],
                            lhsT=shb_sb[:, (d + 2) * 128:(d + 3) * 128],
                            rhs=keep[:, (t + d) * K:(t + d + 1) * K],
                            start=(i == fidx[bank(t)]),
                            stop=(i == lidx[bank(t)]),
                        )
                    # w2 = cnt * se straight from PSUM (one DVE op per wave;
                    # no scalar drain hop in the wave handoff)
                    nc.vector.tensor_mul(
                        w2b[:, wa * K:wb * K],
                        pcn[:, 0:(wb - wa) * K],
                        seb[:, wa * K:wb * K])
                    # VLAD for the wave; dummy absorbs the w2 DVE wait so
                    # the stream matmuls carry only the xvb DMA wait
                    dummy = pp.tile([128, K], f32, tag="dum", bufs=1)
                    nc.tensor.matmul(
                        dummy[:, 0:K],
                        lhsT=shb_sb[:, 0:128],
                        rhs=w2b[:, wa * K:(wa + 1) * K],
                        start=True, stop=True)
                    for t in range(wa, wb):
                        lt = w2b[:, t * K:(t + 1) * K]
                        base = (t - V0) * C1
                        nc.tensor.matmul(
                            pv0[:], lhsT=lt,
                            rhs=xvb_sb[:, base:base + C],
                            start=(t == CNW[0][0]), stop=(t == CNW[-1][1] - 1))
                        nc.tensor.matmul(
                            pv1[:, 0:1], lhsT=lt,
                            rhs=xvb_sb[:, base + C:base + C1],
                            start=(t == CNW[0][0]), stop=(t == CNW[-1][1] - 1))

                # pipeline: wave i's cnt/w2/VLAD is emitted between chunk
                # i+1's keep and its tail so the w2 TT lands early on DVE
                top2_keep(0)
                top2_tail(0)
                for ci in range(1, len(TPB)):
                    top2_keep(ci)
                    wave(ci - 1)
                    top2_tail(ci)
                wave(len(CNW) - 1)

                # ---- drain partials; host sums cores + normalizes
                nc.scalar.copy(vl_sb[:, 0:C], pv0[:])
                nc.scalar.copy(vl_sb[:, C:C1], pv1[:, 0:1])
                nc.sync.dma_start(y[:], vl_sb[:])
    _prune_waits(nc)
    return nc


def _prune_waits(nc):
    """Drop semaphore waits transitively implied by another wait on the same
    instruction (the walrus codegen allows one sync wait per instruction).
    Per-engine queues execute in-order, so a later instruction's completion
    implies every earlier same-engine instruction's waits held (including
    non-sem-updating ones like InstLdweights)."""
    insts = [ins for bb in nc.main_func.blocks for ins in bb.instructions]
    proc_events = {}
    waits_of = {}
    stream_of = {}           # id(ins) -> (engine_key, index)
    stream_cum = {}          # engine_key -> list of cumulative wait-unions
    for ins in insts:
        si = getattr(ins, "sync_info", None)
        if si is None:
            continue
        ow = list(si.on_wait or [])
        waits_of[id(ins)] = [(w.ant_name, w.wait_value) for w in ow]
        eng = str(getattr(ins, "engine", None))
        if eng and "Pool" not in eng:
            lst = stream_cum.setdefault(eng, [])
            cur = dict(lst[-1]) if lst else {}
            for (s2, v2) in waits_of[id(ins)]:
                if cur.get(s2, 0) < v2:
                    cur[s2] = v2
            stream_of[id(ins)] = (eng, len(lst))
            lst.append(cur)
        for u in (si.on_update or []):
            if getattr(u, "update_mode", None) not in ("sem-inc", "sem-add-imm"):
                continue
            lst = proc_events.setdefault(u.ant_name, [])
            prev = lst[-1][0] if lst else 0
            lst.append((prev + (u.update_value or 1), ins))

    import bisect

    def holds(sem, v):
        """Exact transitive closure of thresholds implied by sem >= v."""
        out = {}
        work = [(sem, v)]
        while work:
            s, t = work.pop()
            lst = proc_events.get(s)
            if not lst:
                continue
            ticks = [tk for tk, _ in lst]
            i = bisect.bisect_left(ticks, t)
            if i >= len(lst):
                continue
            implied = {}
            for j in range(i + 1):
                _, ins = lst[j]
                st = stream_of.get(id(ins))
                if st is not None:
                    for (s2, v2) in stream_cum[st[0]][st[1]].items():
                        if implied.get(s2, 0) < v2:
                            implied[s2] = v2
                else:
                    for (s2, v2) in waits_of.get(id(ins), []):
                        if implied.get(s2, 0) < v2:
                            implied[s2] = v2
            for s2, v2 in implied.items():
                if out.get(s2, 0) < v2:
                    out[s2] = v2
                    work.append((s2, v2))
        return out

    own_tick = {}
    for sem, lst in proc_events.items():
        for tick, ins in lst:
            own_tick[(id(ins), sem)] = tick

    pruned = 0
    for ins in insts:
        si = getattr(ins, "sync_info", None)
        if si is None or not si.on_wait or len(si.on_wait) < 2:
            continue
        ow = list(si.on_wait)
        kept = list(ow)
        for w in ow:
            if len(kept) == 1:
                break
            mine = own_tick.get((id(ins), w.ant_name))
            if mine is not None and w.wait_value <= mine - 1:
                kept.remove(w)
                pruned += 1
                continue
            others = [o for o in kept if o is not w]
            for o in others:
                h = holds(o.ant_name, o.wait_value)
                if h.get(w.ant_name, 0) >= w.wait_value:
                    kept.remove(w)
                    pruned += 1
                    break
        si.on_wait = kept
    return pruned


def _host_prep(x, conv_w, centroids):
    from concourse import mybir
    bf16np = mybir.dt.np(mybir.dt.bfloat16)
    fp8np = mybir.dt.np(mybir.dt.float8e4)

    x = np.ascontiguousarray(x, dtype=np.float32)
    L = H * W
    norm = np.sqrt((x.astype(np.float64) ** 2).sum(0))
    inv_norm = (1.0 / np.maximum(norm, 1e-12)).astype(np.float32)  # [H,W]
    xn = x * inv_norm[None]                                  # normalized x
    ii = np.arange(H, dtype=np.float32)
    mi = np.minimum(H - 1 - ii, ii)
    m = np.minimum(mi[:, None], mi[None, :]).astype(np.float32)
    m2 = m * m
    mask4 = m2 * m2                                          # [H,W]

    xpad8 = np.zeros((C, H + 2, W), fp8np)
    xpad8[:, 1:H + 1, :] = xn.astype(fp8np)
    xnb_pad = np.zeros(((H + 2) * W, C), bf16np)             # [Lpad, C] bf16
    xnb_pad[W:(H + 1) * W] = xn.reshape(C, L).T.astype(bf16np)
    sc_pad = np.zeros((H + 2) * W, np.float32)
    sc_pad[W:(H + 1) * W] = mask4.reshape(L)

    cwb = np.ascontiguousarray(
        conv_w.astype(np.float32).T.reshape(CT, 128, K)
        .transpose(1, 0, 2).reshape(128, CT * K)).astype(fp8np)

    # 5 banded 0/1 shift matrices: A_d[p', p] = 1 iff 128d + p' - p in D
    D = {di * W + dj for di in (-1, 0, 1) for dj in (-1, 0, 1)}
    shb = np.zeros((128, 5 * 128), bf16np)
    pp_, p_ = np.meshgrid(np.arange(128), np.arange(128), indexing="ij")
    for j, d in enumerate(range(-2, 3)):
        band = np.isin(128 * d + pp_ - p_, list(D))
        shb[:, j * 128:(j + 1) * 128] = band.astype(bf16np)

    V0, V1 = CNW[0][0], CNW[-1][1]
    NV = V1 - V0

    in_maps = []
    for core in range(M):
        r0 = core * RPC
        sl = slice(r0 * W, (r0 + RPC + 2) * W)               # slab, padded coords
        sc0c = sc_pad[sl].copy()
        sc0c[0:W] = 0.0                                      # halo rows give 0
        sc0c[(RPC + 1) * W:] = 0.0
        # xtb: [128, (t, ct, li)] = xn[ct*128+p, t*128+li] (fp8)
        xs = xpad8[:, r0:r0 + RPC + 2, :].reshape(C, Ls)
        xtb = np.ascontiguousarray(
            xs.reshape(CT, 128, NT, 128).transpose(1, 2, 0, 3)
            .reshape(128, NT * CT * 128))
        # xvb: [128, (t-V0, c)] = xnT[t*128+p, c], ones column; tiles V0..V1
        lo = r0 * W + V0 * 128                               # padded coords
        xv = np.ones((NV * 128, C1), bf16np)
        xv[:, 0:C] = xnb_pad[lo:lo + NV * 128]
        xvb = np.ascontiguousarray(
            xv.reshape(NV, 128, C1).transpose(1, 0, 2).reshape(128, NV * C1))
        in_maps.append({
            "xtb": xtb,
            "xvb": xvb,
            "cwb": cwb,
            "shb": shb,
            "cst": np.ascontiguousarray(sc0c.reshape(NT, 128).T,
                                        dtype=np.float32),
        })
    return in_maps


def _ensure_ntff_hook():
    """Install the axon NTFF profile hook if the image's antenv lacks it."""
    import types
    try:
        from antenv.axon_hooks import get_axon_ntff_profile_hook  # noqa: F401
        return
    except ImportError:
        pass
    if "/root/.axon_site" not in sys.path:
        sys.path.insert(0, "/root/.axon_site")
    from trn_agent_boot.trn_boot import _ntff_profile_via_ctypes
    hook = _ntff_profile_via_ctypes("/opt/axon/libaxon_pjrt.so")
    mod = types.ModuleType("antenv.axon_hooks")
    mod.get_axon_ntff_profile_hook = lambda: hook
    mod.set_axon_ntff_profile_hook = lambda h: None
    import antenv
    antenv.axon_hooks = mod
    sys.modules["antenv.axon_hooks"] = mod


def _install_neff_cache():
    """Cache compiled NEFFs across processes, keyed by BIR content hash."""
    import hashlib
    import shutil
    import concourse.bass2jax as b2j

    orig = b2j.compile_bir_kernel
    if getattr(orig, "_neff_cached", False):
        return

    def cached(bir_json, tmpdir, neff_name="file.neff"):
        h = hashlib.sha256(
            bir_json if isinstance(bir_json, bytes) else bir_json.encode()
        ).hexdigest()[:24]
        cdir = "/tmp/neff_cache"
        os.makedirs(cdir, exist_ok=True)
        cpath = os.path.join(cdir, h + ".neff")
        if os.path.exists(cpath):
            dst = os.path.join(tmpdir, neff_name)
            os.makedirs(tmpdir, exist_ok=True)
            shutil.copy(cpath, dst)
            return dst
        out = orig(bir_json, tmpdir, neff_name=neff_name)
        shutil.copy(out, cpath)
        return out

    cached._neff_cached = True
    b2j.compile_bir_kernel = cached


def kernel(x, conv_w, centroids):
    import concourse.bass_utils as bu
    from concourse.bass_utils import run_bass_kernel_spmd
    _install_neff_cache()
    if TRACE:
        _ensure_ntff_hook()
        bu.upload_artifacts = lambda tmpdir: "local://" + tmpdir

    if "nc" not in _CACHE:
        _CACHE["nc"] = _build_nc()
    nc = _CACHE["nc"]
    in_maps = _host_prep(np.asarray(x), np.asarray(conv_w), np.asarray(centroids))
    res = run_bass_kernel_spmd(nc, in_maps, list(range(M)), trace=TRACE)
    _CACHE["last"] = res
    red = np.zeros((K, C1), np.float32)
    for r in res.results:
        red += np.asarray(r["y"], dtype=np.float32)
    vlad = red[:, :C] - red[:, C:C1] * np.asarray(centroids, np.float32)
    vlad /= np.maximum(np.sqrt((vlad ** 2).sum(1))[:, None], 1e-12)
    v = vlad.reshape(1, K * C)
    v /= np.maximum(np.sqrt((v ** 2).sum()), 1e-12)
    return v.astype(np.float32)


# revision 33
# speedup vs baseline: 1.0121x; 1.0121x over previous
"""NetVLAD (vq_codebook) Trainium2 Bass kernel, 8-way spatially sharded. v3.

Math (same identity as v1/v2):
  xn = x / ||x||_C per location; logits = conv_w @ xn; soft = softmax_K
  fold(unfold(soft) * top2keep) == soft * cnt, cnt = 3x3 box-sum of the
  per-cluster top-2 indicator (border wrap artifacts killed by the
  (min-dist-to-border)^4 mask). vlad = sa2 @ xn.T - rowsum(sa2) * centroids.

v3 strategy (all compute in [L-partition, K-free] layout, no transposes):
  - x is L2-normalized on the host and streamed quantized in BOTH layouts:
    fp8e4m3 [C,L]-tiled for the logits lhsT, bf16 [L,C+1] for the VLAD
    moving operand (+ a ones column that yields rowsum(sa2) for free).
  - logits computed directly into [l-tile, K] PSUM banks, 4 tiles/bank;
    one batched exp per bank (no scale, no accumulator read).
  - softmax sum + top-2 keep on DVE in bf16 chunks; cnt = 3x3 box-sum on
    the PE as 5 banded 128x128 0/1 matmuls per tile, d-outer waves.
  - per-core [K, C+1] partials reduced on host (0.03% of FLOPs).
  - xvb stream chunks are gated behind scalar-engine progress via
    WAR deps (a scalar touch reads the chunk region before the DMA
    writes it) so the Tile scheduler cannot front-run the x load.

Sharding: H=192 rows split 8 ways (24 rows/core + 1 halo row each side).
"""
import os
import sys

sys.path.insert(0, "/opt/trn_rl_repo")
os.environ.setdefault("MYCRO_LOCAL_CACHE", "1")

import numpy as np

C, H, W, K = 512, 192, 192, 64
M = 8                       # cores
RPC = H // M                # 24 rows per core
Ls = (RPC + 2) * W          # 4992 slab locations (incl. 1 halo row each side)
NT = Ls // 128              # 39 l-tiles
CT = C // 128               # 4 c-tiles
C1 = C + 1                  # x columns + ones column
XTG = [6, 9, 11, 13]        # xtb DMA chunk sizes (tiles), staggered completion
TPB = [(0, 8), (8, 20), (20, 30), (30, 39)]     # top2 chunks
CNW = [(1, 6), (6, 18), (18, 28), (28, 38)]     # cnt waves == vlad groups
EB = 4                      # exp batch: tiles per PSUM bank
GATES = {0: 0, 2: 1, 4: 2, 6: 3}  # exp-batch index -> xvb chunk after it

TRACE = False               # set by test.py for profiling runs
_CACHE = {}


def _build_nc():
    import concourse.bass as bass
    import concourse.tile as tile
    from concourse import mybir

    f32 = mybir.dt.float32
    bf16 = mybir.dt.bfloat16
    fp8 = mybir.dt.float8e4
    AF = mybir.ActivationFunctionType
    OP = mybir.AluOpType
    AX = mybir.AxisListType

    NV = CNW[-1][1] - CNW[0][0]        # 37 tiles carried in xvb (skip 0, 38)
    V0 = CNW[0][0]

    nc = bass.Bass()
    xtb = nc.dram_tensor("xtb", [128, NT * CT * 128], fp8, kind="ExternalInput")
    xvb = nc.dram_tensor("xvb", [128, NV * C1], bf16, kind="ExternalInput")
    cwb = nc.dram_tensor("cwb", [128, CT * K], fp8, kind="ExternalInput")
    shb = nc.dram_tensor("shb", [128, 5 * 128], bf16, kind="ExternalInput")
    cst = nc.dram_tensor("cst", [128, NT], f32, kind="ExternalInput")
    y = nc.dram_tensor("y", [K, C1], f32, kind="ExternalOutput")

    xtg = np.cumsum([0] + XTG)

    with tile.TileContext(nc) as tc:
        with tc.tile_pool(name="big", bufs=1) as big:
            xtb_sb = big.tile([128, NT * CT * 128], fp8, tag="xtb")
            xvb_sb = big.tile([128, NV * C1], bf16, tag="xvb")
            cwb_sb = big.tile([128, CT * K], fp8, tag="cwb")
            shb_sb = big.tile([128, 5 * 128], bf16, tag="shb")
            sc0 = big.tile([128, NT], f32, tag="sc0")
            expb = big.tile([128, NT * K], bf16, tag="expb")
            keep = big.tile([128, NT * K], bf16, tag="keep")
            seb = big.tile([128, NT * K], bf16, tag="seb")
            w2b = big.tile([128, NT * K], bf16, tag="w2b")
            sume = big.tile([128, NT], f32, tag="sume")
            isum = big.tile([128, NT], f32, tag="isum")
            scc = big.tile([128, NT], f32, tag="scc")
            m8 = big.tile([128, NT * 8], bf16, tag="m8")
            vl_sb = big.tile([K, C1], f32, tag="vl")
            scr = big.tile([128, 4], f32, tag="scr")

            # cwb (needed by the first warmup + phase 1) goes first on the
            # sync DGE; the other constants ride the scalar HWDGE so the
            # xtb stream descriptors start generating immediately after
            nc.sync.dma_start(cwb_sb[:], cwb[:])
            nc.scalar.dma_start(shb_sb[:], shb[:])
            nc.scalar.dma_start(sc0[:], cst[:])
            for g in range(len(XTG)):
                a, b = int(xtg[g]) * CT * 128, int(xtg[g + 1]) * CT * 128
                nc.sync.dma_start(xtb_sb[:, a:b], xtb[:, a:b])

            # single-wait touch absorbing the cst DMA for later DVE/scalar use
            nc.scalar.copy(scr[:, 0:1], sc0[:, 0:1])
            # init the xvb gate columns so the gate touches read defined data
            for (va, _vb) in CNW:
                a = (va - V0) * C1
                nc.vector.memset(xvb_sb[:, a:a + 1], 0.0)

            with tc.tile_pool(name="pp", bufs=1, space="PSUM") as pp:
                # all PSUM tiles are whole-bank (2048B) multiples so every
                # allocation is bank-aligned and matmuls never cross banks:
                # pcn 2 + pv0 1 + pv1 1 + plg 3 + dum 1 = 8 banks
                pcn = pp.tile([128, 2 * C], f32, tag="pcn", bufs=1)
                pv0 = pp.tile([K, C], f32, tag="pv0", bufs=1)
                pv1 = pp.tile([K, C], f32, tag="pv1", bufs=1)
                plg0 = pp.tile([128, C], f32, tag="plg", bufs=3,
                               name="plg0")  # reserve bank-aligned slots

                # warm-up burst: absorbs cwb/shb DMA waits, spins up the HAM
                dummy = pp.tile([128, K], f32, tag="dum", bufs=1)
                nc.tensor.matmul(dummy[0:64, 0:64], lhsT=cwb_sb[:, 0:64],
                                 rhs=cwb_sb[:, 0:64], start=True, stop=True)
                for _ in range(6):
                    dummy = pp.tile([128, K], f32, tag="dum", bufs=1)
                    nc.tensor.matmul(dummy[:, 0:64], lhsT=shb_sb[:, 0:128],
                                     rhs=shb_sb[:, 0:64], start=True, stop=True)

                # ---- phase 1: logits into [l-tile, K] PSUM, 4 tiles/bank;
                # batched exp per bank (x pre-normalized, so no scale)
                nb = (NT + EB - 1) // EB
                gate_cols = {}
                plogs = {}
                cg = 0                      # xtb chunk cursor

                DR = mybir.MatmulPerfMode.DoubleRow
                for bi in range(nb):
                    t0, t1 = bi * EB, min(NT, (bi + 1) * EB)
                    plg = pp.tile([128, C], f32, tag="plg", bufs=3)
                    plogs[bi] = plg
                    for t in range(t0, t1):
                        if cg < len(XTG) and t == int(xtg[cg]):
                            # dummy matmul absorbs this xtb chunk's DMA wait
                            dummy = pp.tile([128, K], f32, tag="dum", bufs=1)
                            base = t * CT * 128
                            nc.tensor.matmul(
                                dummy[:, 0:64],
                                lhsT=xtb_sb[:, base:base + 128],
                                rhs=xtb_sb[:, base:base + 64],
                                start=True, stop=True)
                            cg += 1
                        # fp8 DoubleRow: 256-deep contraction per matmul,
                        # c-halves stacked in the AP middle dim
                        for h in range(2):
                            lv = xtb_sb[:, (t * CT + 2 * h) * 128:
                                        (t * CT + 2 * h + 2) * 128]
                            rv = cwb_sb[:, 2 * h * K:(2 * h + 2) * K]
                            nc.tensor.matmul(
                                plg[:, (t - t0) * K:(t - t0 + 1) * K],
                                lhsT=lv.rearrange("p (two f) -> p two f",
                                                  two=2),
                                rhs=rv.rearrange("p (two f) -> p two f",
                                                 two=2),
                                start=(h == 0), stop=(h == 1),
                                perf_mode=DR,
                            )
                    nc.scalar.activation(
                        expb[:, t0 * K:t1 * K], plg[:, 0:(t1 - t0) * K],
                        AF.Exp)
                    if bi in GATES:
                        # gate: touch-read the xvb chunk region, then issue
                        # its DMA; the WAR dep paces the stream off scalar
                        # progress (the Tile scheduler keeps DMA after touch)
                        gi = GATES[bi]
                        va, vb = CNW[gi]
                        a = (va - V0) * C1
                        b = (vb - V0) * C1
                        nc.scalar.copy(scr[:, 2:3], xvb_sb[:, a:a + 1])
                        nc.sync.dma_start(xvb_sb[:, a:b], xvb[:, a:b])

                # ---- phases 2-4, software-pipelined: top2 chunk ci, then
                # wave ci-1's [cnt matmuls -> scalar drain -> w2 -> VLAD]
                # (wave i only needs keep from chunk i, so trailing by one
                # chunk keeps every engine busy without stalls)

                def top2_keep(ci):
                    ta, tb = TPB[ci]
                    n = tb - ta
                    e3 = expb[:, ta * K:tb * K].rearrange(
                        "p (t k) -> p t k", k=K)
                    k3 = keep[:, ta * K:tb * K].rearrange(
                        "p (t k) -> p t k", k=K)
                    # per-tile top-8 (Max8); m2 = column 1
                    for t in range(ta, tb):
                        nc.vector.max(m8[:, t * 8:(t + 1) * 8],
                                      expb[:, t * K:(t + 1) * K])
                    m83 = m8[:, ta * 8:tb * 8].rearrange(
                        "p (t e) -> p t e", e=8)
                    m2c = m83[:, :, 1:2].broadcast_to([128, n, K])
                    nc.vector.tensor_tensor(k3, e3, m2c, op=OP.is_ge)

                def top2_tail(ci):
                    ta, tb = TPB[ci]
                    n = tb - ta
                    e3 = expb[:, ta * K:tb * K].rearrange(
                        "p (t k) -> p t k", k=K)
                    s3 = seb[:, ta * K:tb * K].rearrange(
                        "p (t k) -> p t k", k=K)
                    scc_c = scc[:, ta:tb][:, :, None].broadcast_to([128, n, K])
                    nc.vector.tensor_reduce(
                        sume[:, ta:tb], e3, axis=AX.X, op=OP.add)
                    nc.vector.reciprocal(isum[:, ta:tb], sume[:, ta:tb])
                    nc.vector.tensor_mul(
                        scc[:, ta:tb], sc0[:, ta:tb], isum[:, ta:tb])
                    nc.vector.tensor_tensor(s3, e3, scc_c, op=OP.mult)

                def wave(wi):
                    wa, wb = CNW[wi]
                    # dummy absorbs this wave's keep (DVE) wait
                    dummy = pp.tile([128, K], f32, tag="dum", bufs=1)
                    nc.tensor.matmul(
                        dummy[:, 0:K],
                        lhsT=shb_sb[:, 0:128],
                        rhs=keep[:, wa * K:(wa + 1) * K],
                        start=True, stop=True)
                    # d-outer banded matmuls: one LDWEIGHTS per shift matrix.
                    # start/stop once per 2KB PSUM zero region (8 tiles/bank):
                    # start=True lazily zeroes the bank; first-touch writes of
                    # other tiles store, later touches accumulate.
                    pairs = [(d, t) for d in range(-2, 3)
                             for t in range(wa, wb) if 0 <= t + d < NT]
                    bank = lambda t: (t - wa) // 8
                    fidx = {}
                    lidx = {}
                    for i, (d, t) in enumerate(pairs):
                        fidx.setdefault(bank(t), i)
                        lidx[bank(t)] = i
                    for i, (d, t) in enumerate(pairs):
                        nc.tensor.matmul(
                            pcn[:, (t - wa) * K:(t - wa + 1) * K],
                            lhsT=shb_sb[:, (d + 2) * 128:(d + 3) * 128],
                            rhs=keep[:, (t + d) * K:(t + d + 1) * K],
                            start=(i == fidx[bank(t)]),
                            stop=(i == lidx[bank(t)]),
                        )
                    # w2 = cnt * se straight from PSUM (one DVE op per wave;
                    # no scalar drain hop in the wave handoff)
                    nc.vector.tensor_mul(
                        w2b[:, wa * K:wb * K],
                        pcn[:, 0:(wb - wa) * K],
                        seb[:, wa * K:wb * K])
                    # VLAD for the wave; dummy absorbs the w2 DVE wait so
                    # the stream matmuls carry only the xvb DMA wait
                    dummy = pp.tile([128, K], f32, tag="dum", bufs=1)
                    nc.tensor.matmul(
                        dummy[:, 0:K],
                        lhsT=shb_sb[:, 0:128],
                        rhs=w2b[:, wa * K:(wa + 1) * K],
                        start=True, stop=True)
                    for t in range(wa, wb):
                        lt = w2b[:, t * K:(t + 1) * K]
                        base = (t - V0) * C1
                        nc.tensor.matmul(
                            pv0[:], lhsT=lt,
                            rhs=xvb_sb[:, base:base + C],
                            start=(t == CNW[0][0]), stop=(t == CNW[-1][1] - 1))
                        nc.tensor.matmul(
                            pv1[:, 0:1], lhsT=lt,
                            rhs=xvb_sb[:, base + C:base + C1],
                            start=(t == CNW[0][0]), stop=(t == CNW[-1][1] - 1))

                # pipeline: wave i's cnt/w2/VLAD is emitted between chunk
                # i+1's keep and its tail so the w2 TT lands early on DVE
                top2_keep(0)
                top2_tail(0)
                for ci in range(1, len(TPB)):
                    top2_keep(ci)
                    wave(ci - 1)
                    top2_tail(ci)
                wave(len(CNW) - 1)

                # ---- drain partials; host sums cores + normalizes
                nc.scalar.copy(vl_sb[:, 0:C], pv0[:])
                nc.scalar.copy(vl_sb[:, C:C1], pv1[:, 0:1])
                nc.sync.dma_start(y[:], vl_sb[:])
    _prune_waits(nc)
    return nc


def _prune_waits(nc):
    """Drop semaphore waits transitively implied by another wait on the same
    instruction (the walrus codegen allows one sync wait per instruction).
    Per-engine queues execute in-order, so a later instruction's completion
    implies every earlier same-engine instruction's waits held (including
    non-sem-updating ones like InstLdweights)."""
    insts = [ins for bb in nc.main_func.blocks for ins in bb.instructions]
    proc_events = {}
    waits_of = {}
    stream_of = {}           # id(ins) -> (engine_key, index)
    stream_cum = {}          # engine_key -> list of cumulative wait-unions
    for ins in insts:
        si = getattr(ins, "sync_info", None)
        if si is None:
            continue
        ow = list(si.on_wait or [])
        waits_of[id(ins)] = [(w.ant_name, w.wait_value) for w in ow]
        eng = str(getattr(ins, "engine", None))
        if eng and "Pool" not in eng:
            lst = stream_cum.setdefault(eng, [])
            cur = dict(lst[-1]) if lst else {}
            for (s2, v2) in waits_of[id(ins)]:
                if cur.get(s2, 0) < v2:
                    cur[s2] = v2
            stream_of[id(ins)] = (eng, len(lst))
            lst.append(cur)
        for u in (si.on_update or []):
            if getattr(u, "update_mode", None) not in ("sem-inc", "sem-add-imm"):
                continue
            lst = proc_events.setdefault(u.ant_name, [])
            prev = lst[-1][0] if lst else 0
            lst.append((prev + (u.update_value or 1), ins))

    import bisect

    def holds(sem, v):
        """Exact transitive closure of thresholds implied by sem >= v."""
        out = {}
        work = [(sem, v)]
        while work:
            s, t = work.pop()
            lst = proc_events.get(s)
            if not lst:
                continue
            ticks = [tk for tk, _ in lst]
            i = bisect.bisect_left(ticks, t)
            if i >= len(lst):
                continue
            implied = {}
            for j in range(i + 1):
                _, ins = lst[j]
                st = stream_of.get(id(ins))
                if st is not None:
                    for (s2, v2) in stream_cum[st[0]][st[1]].items():
                        if implied.get(s2, 0) < v2:
                            implied[s2] = v2
                else:
                    for (s2, v2) in waits_of.get(id(ins), []):
                        if implied.get(s2, 0) < v2:
                            implied[s2] = v2
            for s2, v2 in implied.items():
                if out.get(s2, 0) < v2:
                    out[s2] = v2
                    work.append((s2, v2))
        return out

    own_tick = {}
    for sem, lst in proc_events.items():
        for tick, ins in lst:
            own_tick[(id(ins), sem)] = tick

    pruned = 0
    for ins in insts:
        si = getattr(ins, "sync_info", None)
        if si is None or not si.on_wait or len(si.on_wait) < 2:
            continue
        ow = list(si.on_wait)
        kept = list(ow)
        for w in ow:
            if len(kept) == 1:
                break
            mine = own_tick.get((id(ins), w.ant_name))
            if mine is not None and w.wait_value <= mine - 1:
                kept.remove(w)
                pruned += 1
                continue
            others = [o for o in kept if o is not w]
            for o in others:
                h = holds(o.ant_name, o.wait_value)
                if h.get(w.ant_name, 0) >= w.wait_value:
                    kept.remove(w)
                    pruned += 1
                    break
        si.on_wait = kept
    return pruned


def _host_prep(x, conv_w, centroids):
    from concourse import mybir
    bf16np = mybir.dt.np(mybir.dt.bfloat16)
    fp8np = mybir.dt.np(mybir.dt.float8e4)

    x = np.ascontiguousarray(x, dtype=np.float32)
    L = H * W
    norm = np.sqrt((x.astype(np.float64) ** 2).sum(0))
    inv_norm = (1.0 / np.maximum(norm, 1e-12)).astype(np.float32)  # [H,W]
    xn = x * inv_norm[None]                                  # normalized x
    ii = np.arange(H, dtype=np.float32)
    mi = np.minimum(H - 1 - ii, ii)
    m = np.minimum(mi[:, None], mi[None, :]).astype(np.float32)
    m2 = m * m
    mask4 = m2 * m2                                          # [H,W]

    xpad8 = np.zeros((C, H + 2, W), fp8np)
    xpad8[:, 1:H + 1, :] = xn.astype(fp8np)
    xnb_pad = np.zeros(((H + 2) * W, C), bf16np)             # [Lpad, C] bf16
    xnb_pad[W:(H + 1) * W] = xn.reshape(C, L).T.astype(bf16np)
    sc_pad = np.zeros((H + 2) * W, np.float32)
    sc_pad[W:(H + 1) * W] = mask4.reshape(L)

    cwb = np.ascontiguousarray(
        conv_w.astype(np.float32).T.reshape(CT, 128, K)
        .transpose(1, 0, 2).reshape(128, CT * K)).astype(fp8np)

    # 5 banded 0/1 shift matrices: A_d[p', p] = 1 iff 128d + p' - p in D
    D = {di * W + dj for di in (-1, 0, 1) for dj in (-1, 0, 1)}
    shb = np.zeros((128, 5 * 128), bf16np)
    pp_, p_ = np.meshgrid(np.arange(128), np.arange(128), indexing="ij")
    for j, d in enumerate(range(-2, 3)):
        band = np.isin(128 * d + pp_ - p_, list(D))
        shb[:, j * 128:(j + 1) * 128] = band.astype(bf16np)

    V0, V1 = CNW[0][0], CNW[-1][1]
    NV = V1 - V0

    in_maps = []
    for core in range(M):
        r0 = core * RPC
        sl = slice(r0 * W, (r0 + RPC + 2) * W)               # slab, padded coords
        sc0c = sc_pad[sl].copy()
        sc0c[0:W] = 0.0                                      # halo rows give 0
        sc0c[(RPC + 1) * W:] = 0.0
        # xtb: [128, (t, ct, li)] = xn[ct*128+p, t*128+li] (fp8)
        xs = xpad8[:, r0:r0 + RPC + 2, :].reshape(C, Ls)
        xtb = np.ascontiguousarray(
            xs.reshape(CT, 128, NT, 128).transpose(1, 2, 0, 3)
            .reshape(128, NT * CT * 128))
        # xvb: [128, (t-V0, c)] = xnT[t*128+p, c], ones column; tiles V0..V1
        lo = r0 * W + V0 * 128                               # padded coords
        xv = np.ones((NV * 128, C1), bf16np)
        xv[:, 0:C] = xnb_pad[lo:lo + NV * 128]
        xvb = np.ascontiguousarray(
            xv.reshape(NV, 128, C1).transpose(1, 0, 2).reshape(128, NV * C1))
        in_maps.append({
            "xtb": xtb,
            "xvb": xvb,
            "cwb": cwb,
            "shb": shb,
            "cst": np.ascontiguousarray(sc0c.reshape(NT, 128).T,
                                        dtype=np.float32),
        })
    return in_maps


def _ensure_ntff_hook():
    """Install the axon NTFF profile hook if the image's antenv lacks it."""
    import types
    try:
        from antenv.axon_hooks import get_axon_ntff_profile_hook  # noqa: F401
        return
    except ImportError:
        pass
    if "/root/.axon_site" not in sys.path:
        sys.path.insert(0, "/root/.axon_site")
    from trn_agent_boot.trn_boot import _ntff_profile_via_ctypes
    hook = _ntff_profile_via_ctypes("/opt/axon/libaxon_pjrt.so")
    mod = types.ModuleType("antenv.axon_hooks")
    mod.get_axon_ntff_profile_hook = lambda: hook
    mod.set_axon_ntff_profile_hook = lambda h: None
    import antenv
    antenv.axon_hooks = mod
    sys.modules["antenv.axon_hooks"] = mod


def _install_neff_cache():
    """Cache compiled NEFFs across processes, keyed by BIR content hash."""
    import hashlib
    import shutil
    import concourse.bass2jax as b2j

    orig = b2j.compile_bir_kernel
    if getattr(orig, "_neff_cached", False):
        return

    def cached(bir_json, tmpdir, neff_name="file.neff"):
        h = hashlib.sha256(
            bir_json if isinstance(bir_json, bytes) else bir_json.encode()
        ).hexdigest()[:24]
        cdir = "/tmp/neff_cache"
        os.makedirs(cdir, exist_ok=True)
        cpath = os.path.join(cdir, h + ".neff")
        if os.path.exists(cpath):
            dst = os.path.join(tmpdir, neff_name)
            os.makedirs(tmpdir, exist_ok=True)
            shutil.copy(cpath, dst)
            return dst
        out = orig(bir_json, tmpdir, neff_name=neff_name)
        shutil.copy(out, cpath)
        return out

    cached._neff_cached = True
    b2j.compile_bir_kernel = cached


def kernel(x, conv_w, centroids):
    import concourse.bass_utils as bu
    from concourse.bass_utils import run_bass_kernel_spmd
    _install_neff_cache()
    if TRACE:
        _ensure_ntff_hook()
        bu.upload_artifacts = lambda tmpdir: "local://" + tmpdir

    if "nc" not in _CACHE:
        _CACHE["nc"] = _build_nc()
    nc = _CACHE["nc"]
    in_maps = _host_prep(np.asarray(x), np.asarray(conv_w), np.asarray(centroids))
    res = run_bass_kernel_spmd(nc, in_maps, list(range(M)), trace=TRACE)
    _CACHE["last"] = res
    red = np.zeros((K, C1), np.float32)
    for r in res.results:
        red += np.asarray(r["y"], dtype=np.float32)
    vlad = red[:, :C] - red[:, C:C1] * np.asarray(centroids, np.float32)
    vlad /= np.maximum(np.sqrt((vlad ** 2).sum(1))[:, None], 1e-12)
    v = vlad.reshape(1, K * C)
    v /= np.maximum(np.sqrt((v ** 2).sum()), 1e-12)
    return v.astype(np.float32)


# revision 34
# speedup vs baseline: 1.0522x; 1.0397x over previous
"""NetVLAD (vq_codebook) Trainium2 Bass kernel, 8-way spatially sharded. v3.

Math (same identity as v1/v2):
  xn = x / ||x||_C per location; logits = conv_w @ xn; soft = softmax_K
  fold(unfold(soft) * top2keep) == soft * cnt, cnt = 3x3 box-sum of the
  per-cluster top-2 indicator (border wrap artifacts killed by the
  (min-dist-to-border)^4 mask). vlad = sa2 @ xn.T - rowsum(sa2) * centroids.

v3 strategy (all compute in [L-partition, K-free] layout, no transposes):
  - x is L2-normalized on the host and streamed quantized in BOTH layouts:
    fp8e4m3 [C,L]-tiled for the logits lhsT, bf16 [L,C+1] for the VLAD
    moving operand (+ a ones column that yields rowsum(sa2) for free).
  - logits computed directly into [l-tile, K] PSUM banks, 4 tiles/bank;
    one batched exp per bank (no scale, no accumulator read).
  - softmax sum + top-2 keep on DVE in bf16 chunks; cnt = 3x3 box-sum on
    the PE as 5 banded 128x128 0/1 matmuls per tile, d-outer waves.
  - per-core [K, C+1] partials reduced on host (0.03% of FLOPs).
  - xvb stream chunks are gated behind scalar-engine progress via
    WAR deps (a scalar touch reads the chunk region before the DMA
    writes it) so the Tile scheduler cannot front-run the x load.

Sharding: H=192 rows split 8 ways (24 rows/core + 1 halo row each side).
"""
import os
import sys

sys.path.insert(0, "/opt/trn_rl_repo")
os.environ.setdefault("MYCRO_LOCAL_CACHE", "1")

import numpy as np

C, H, W, K = 512, 192, 192, 64
M = 8                       # cores
RPC = H // M                # 24 rows per core
Ls = (RPC + 2) * W          # 4992 slab locations (incl. 1 halo row each side)
NT = Ls // 128              # 39 l-tiles
CT = C // 128               # 4 c-tiles
C1 = C + 1                  # x columns + ones column
XTG = [6, 9, 11, 13]        # xtb DMA chunk sizes (tiles), staggered completion
TPB = [(0, 8), (8, 20), (20, 30), (30, 39)]     # top2 chunks
CNW = [(1, 6), (6, 18), (18, 28), (28, 38)]     # cnt waves == vlad groups
EB = 8                      # exp batch: tiles per PSUM bank (batch 0 ==
                            # top2 chunk 0, so the keep chain starts early)
GATES = {0: 0, 1: 1, 2: 2, 3: 3}  # exp-batch index -> xvb chunk after it

TRACE = False               # set by test.py for profiling runs
_CACHE = {}


def _build_nc():
    import concourse.bass as bass
    import concourse.tile as tile
    from concourse import mybir

    f32 = mybir.dt.float32
    bf16 = mybir.dt.bfloat16
    fp8 = mybir.dt.float8e4
    AF = mybir.ActivationFunctionType
    OP = mybir.AluOpType
    AX = mybir.AxisListType

    NV = CNW[-1][1] - CNW[0][0]        # 37 tiles carried in xvb (skip 0, 38)
    V0 = CNW[0][0]

    nc = bass.Bass()
    xtb = nc.dram_tensor("xtb", [128, NT * CT * 128], fp8, kind="ExternalInput")
    xvb = nc.dram_tensor("xvb", [128, NV * C1], bf16, kind="ExternalInput")
    cwb = nc.dram_tensor("cwb", [128, CT * K], fp8, kind="ExternalInput")
    shb = nc.dram_tensor("shb", [128, 5 * 128], bf16, kind="ExternalInput")
    cst = nc.dram_tensor("cst", [128, NT], f32, kind="ExternalInput")
    y = nc.dram_tensor("y", [K, C1], f32, kind="ExternalOutput")

    xtg = np.cumsum([0] + XTG)

    with tile.TileContext(nc) as tc:
        with tc.tile_pool(name="big", bufs=1) as big:
            xtb_sb = big.tile([128, NT * CT * 128], fp8, tag="xtb")
            xvb_sb = big.tile([128, NV * C1], bf16, tag="xvb")
            cwb_sb = big.tile([128, CT * K], fp8, tag="cwb")
            shb_sb = big.tile([128, 5 * 128], bf16, tag="shb")
            sc0 = big.tile([128, NT], f32, tag="sc0")
            expb = big.tile([128, NT * K], bf16, tag="expb")
            keep = big.tile([128, NT * K], bf16, tag="keep")
            seb = big.tile([128, NT * K], bf16, tag="seb")
            w2b = big.tile([128, NT * K], bf16, tag="w2b")
            sume = big.tile([128, NT], f32, tag="sume")
            isum = big.tile([128, NT], f32, tag="isum")
            scc = big.tile([128, NT], f32, tag="scc")
            m8 = big.tile([128, NT * 8], bf16, tag="m8")
            vl_sb = big.tile([K, C1], f32, tag="vl")
            scr = big.tile([128, 4], f32, tag="scr")

            # cwb (needed by the first warmup + phase 1) goes first on the
            # sync DGE; the other constants ride the scalar HWDGE so the
            # xtb stream descriptors start generating immediately after
            nc.sync.dma_start(cwb_sb[:], cwb[:])
            nc.scalar.dma_start(shb_sb[:], shb[:])
            nc.scalar.dma_start(sc0[:], cst[:])
            for g in range(len(XTG)):
                a, b = int(xtg[g]) * CT * 128, int(xtg[g + 1]) * CT * 128
                nc.sync.dma_start(xtb_sb[:, a:b], xtb[:, a:b])

            # single-wait touch absorbing the cst DMA for later DVE/scalar use
            nc.scalar.copy(scr[:, 0:1], sc0[:, 0:1])
            # init the xvb gate columns so the gate touches read defined data
            for (va, _vb) in CNW:
                a = (va - V0) * C1
                nc.vector.memset(xvb_sb[:, a:a + 1], 0.0)

            with tc.tile_pool(name="pp", bufs=1, space="PSUM") as pp:
                # all PSUM tiles are whole-bank (2048B) multiples so every
                # allocation is bank-aligned and matmuls never cross banks:
                # pcn 2 + pv0 1 + pv1 1 + plg 3 + dum 1 = 8 banks
                pcn = pp.tile([128, 2 * C], f32, tag="pcn", bufs=1)
                pv0 = pp.tile([K, C], f32, tag="pv0", bufs=1)
                pv1 = pp.tile([K, C], f32, tag="pv1", bufs=1)
                plg0 = pp.tile([128, C], f32, tag="plg", bufs=3,
                               name="plg0")  # reserve bank-aligned slots

                # warm-up burst: absorbs cwb/shb DMA waits, spins up the HAM
                dummy = pp.tile([128, K], f32, tag="dum", bufs=1)
                nc.tensor.matmul(dummy[0:64, 0:64], lhsT=cwb_sb[:, 0:64],
                                 rhs=cwb_sb[:, 0:64], start=True, stop=True)
                for _ in range(6):
                    dummy = pp.tile([128, K], f32, tag="dum", bufs=1)
                    nc.tensor.matmul(dummy[:, 0:64], lhsT=shb_sb[:, 0:128],
                                     rhs=shb_sb[:, 0:64], start=True, stop=True)

                # ---- phase 1: logits into [l-tile, K] PSUM, 4 tiles/bank;
                # batched exp per bank (x pre-normalized, so no scale)
                nb = (NT + EB - 1) // EB
                gate_cols = {}
                plogs = {}
                cg = 0                      # xtb chunk cursor

                DR = mybir.MatmulPerfMode.DoubleRow
                for bi in range(nb):
                    t0, t1 = bi * EB, min(NT, (bi + 1) * EB)
                    plg = pp.tile([128, C], f32, tag="plg", bufs=3)
                    plogs[bi] = plg
                    for t in range(t0, t1):
                        if cg < len(XTG) and t == int(xtg[cg]):
                            # dummy matmul absorbs this xtb chunk's DMA wait
                            dummy = pp.tile([128, K], f32, tag="dum", bufs=1)
                            base = t * CT * 128
                            nc.tensor.matmul(
                                dummy[:, 0:64],
                                lhsT=xtb_sb[:, base:base + 128],
                                rhs=xtb_sb[:, base:base + 64],
                                start=True, stop=True)
                            cg += 1
                        # fp8 DoubleRow: 256-deep contraction per matmul,
                        # c-halves stacked in the AP middle dim
                        for h in range(2):
                            lv = xtb_sb[:, (t * CT + 2 * h) * 128:
                                        (t * CT + 2 * h + 2) * 128]
                            rv = cwb_sb[:, 2 * h * K:(2 * h + 2) * K]
                            nc.tensor.matmul(
                                plg[:, (t - t0) * K:(t - t0 + 1) * K],
                                lhsT=lv.rearrange("p (two f) -> p two f",
                                                  two=2),
                                rhs=rv.rearrange("p (two f) -> p two f",
                                                 two=2),
                                start=(h == 0), stop=(h == 1),
                                perf_mode=DR,
                            )
                    nc.scalar.activation(
                        expb[:, t0 * K:t1 * K], plg[:, 0:(t1 - t0) * K],
                        AF.Exp)
                    if bi in GATES:
                        # gate: touch-read the xvb chunk region, then issue
                        # its DMA; the WAR dep paces the stream off scalar
                        # progress (the Tile scheduler keeps DMA after touch)
                        gi = GATES[bi]
                        va, vb = CNW[gi]
                        a = (va - V0) * C1
                        b = (vb - V0) * C1
                        nc.scalar.copy(scr[:, 2:3], xvb_sb[:, a:a + 1])
                        nc.sync.dma_start(xvb_sb[:, a:b], xvb[:, a:b])

                # ---- phases 2-4, software-pipelined: top2 chunk ci, then
                # wave ci-1's [cnt matmuls -> scalar drain -> w2 -> VLAD]
                # (wave i only needs keep from chunk i, so trailing by one
                # chunk keeps every engine busy without stalls)

                def top2_keep(ci):
                    ta, tb = TPB[ci]
                    n = tb - ta
                    e3 = expb[:, ta * K:tb * K].rearrange(
                        "p (t k) -> p t k", k=K)
                    k3 = keep[:, ta * K:tb * K].rearrange(
                        "p (t k) -> p t k", k=K)
                    # per-tile top-8 (Max8); m2 = column 1
                    for t in range(ta, tb):
                        nc.vector.max(m8[:, t * 8:(t + 1) * 8],
                                      expb[:, t * K:(t + 1) * K])
                    m83 = m8[:, ta * 8:tb * 8].rearrange(
                        "p (t e) -> p t e", e=8)
                    m2c = m83[:, :, 1:2].broadcast_to([128, n, K])
                    nc.vector.tensor_tensor(k3, e3, m2c, op=OP.is_ge)

                def top2_tail(ci):
                    ta, tb = TPB[ci]
                    n = tb - ta
                    e3 = expb[:, ta * K:tb * K].rearrange(
                        "p (t k) -> p t k", k=K)
                    s3 = seb[:, ta * K:tb * K].rearrange(
                        "p (t k) -> p t k", k=K)
                    scc_c = scc[:, ta:tb][:, :, None].broadcast_to([128, n, K])
                    nc.vector.tensor_reduce(
                        sume[:, ta:tb], e3, axis=AX.X, op=OP.add)
                    nc.vector.reciprocal(isum[:, ta:tb], sume[:, ta:tb])
                    nc.vector.tensor_mul(
                        scc[:, ta:tb], sc0[:, ta:tb], isum[:, ta:tb])
                    nc.vector.tensor_tensor(s3, e3, scc_c, op=OP.mult)

                def wave(wi):
                    wa, wb = CNW[wi]
                    # dummy absorbs this wave's keep (DVE) wait
                    dummy = pp.tile([128, K], f32, tag="dum", bufs=1)
                    nc.tensor.matmul(
                        dummy[:, 0:K],
                        lhsT=shb_sb[:, 0:128],
                        rhs=keep[:, wa * K:(wa + 1) * K],
                        start=True, stop=True)
                    # d-outer banded matmuls: one LDWEIGHTS per shift matrix.
                    # start/stop once per 2KB PSUM zero region (8 tiles/bank):
                    # start=True lazily zeroes the bank; first-touch writes of
                    # other tiles store, later touches accumulate.
                    pairs = [(d, t) for d in range(-2, 3)
                             for t in range(wa, wb) if 0 <= t + d < NT]
                    bank = lambda t: (t - wa) // 8
                    fidx = {}
                    lidx = {}
                    for i, (d, t) in enumerate(pairs):
                        fidx.setdefault(bank(t), i)
                        lidx[bank(t)] = i
                    for i, (d, t) in enumerate(pairs):
                        nc.tensor.matmul(
                            pcn[:, (t - wa) * K:(t - wa + 1) * K],
                            lhsT=shb_sb[:, (d + 2) * 128:(d + 3) * 128],
                            rhs=keep[:, (t + d) * K:(t + d + 1) * K],
                            start=(i == fidx[bank(t)]),
                            stop=(i == lidx[bank(t)]),
                        )
                    # w2 = cnt * se straight from PSUM (one DVE op per wave;
                    # no scalar drain hop in the wave handoff)
                    nc.vector.tensor_mul(
                        w2b[:, wa * K:wb * K],
                        pcn[:, 0:(wb - wa) * K],
                        seb[:, wa * K:wb * K])
                    # VLAD for the wave; dummy absorbs the w2 DVE wait so
                    # the stream matmuls carry only the xvb DMA wait
                    dummy = pp.tile([128, K], f32, tag="dum", bufs=1)
                    nc.tensor.matmul(
                        dummy[:, 0:K],
                        lhsT=shb_sb[:, 0:128],
                        rhs=w2b[:, wa * K:(wa + 1) * K],
                        start=True, stop=True)
                    for t in range(wa, wb):
                        lt = w2b[:, t * K:(t + 1) * K]
                        base = (t - V0) * C1
                        nc.tensor.matmul(
                            pv0[:], lhsT=lt,
                            rhs=xvb_sb[:, base:base + C],
                            start=(t == CNW[0][0]), stop=(t == CNW[-1][1] - 1))
                        nc.tensor.matmul(
                            pv1[:, 0:1], lhsT=lt,
                            rhs=xvb_sb[:, base + C:base + C1],
                            start=(t == CNW[0][0]), stop=(t == CNW[-1][1] - 1))

                # pipeline: wave i's cnt/w2/VLAD is emitted between chunk
                # i+1's keep and its tail so the w2 TT lands early on DVE
                top2_keep(0)
                top2_tail(0)
                for ci in range(1, len(TPB)):
                    top2_keep(ci)
                    wave(ci - 1)
                    top2_tail(ci)
                wave(len(CNW) - 1)

                # ---- drain partials; host sums cores + normalizes
                nc.scalar.copy(vl_sb[:, 0:C], pv0[:])
                nc.scalar.copy(vl_sb[:, C:C1], pv1[:, 0:1])
                nc.sync.dma_start(y[:], vl_sb[:])
    _prune_waits(nc)
    return nc


def _prune_waits(nc):
    """Drop semaphore waits transitively implied by another wait on the same
    instruction (the walrus codegen allows one sync wait per instruction).
    Per-engine queues execute in-order, so a later instruction's completion
    implies every earlier same-engine instruction's waits held (including
    non-sem-updating ones like InstLdweights)."""
    insts = [ins for bb in nc.main_func.blocks for ins in bb.instructions]
    proc_events = {}
    waits_of = {}
    stream_of = {}           # id(ins) -> (engine_key, index)
    stream_cum = {}          # engine_key -> list of cumulative wait-unions
    for ins in insts:
        si = getattr(ins, "sync_info", None)
        if si is None:
            continue
        ow = list(si.on_wait or [])
        waits_of[id(ins)] = [(w.ant_name, w.wait_value) for w in ow]
        eng = str(getattr(ins, "engine", None))
        if eng and "Pool" not in eng:
            lst = stream_cum.setdefault(eng, [])
            cur = dict(lst[-1]) if lst else {}
            for (s2, v2) in waits_of[id(ins)]:
                if cur.get(s2, 0) < v2:
                    cur[s2] = v2
            stream_of[id(ins)] = (eng, len(lst))
            lst.append(cur)
        for u in (si.on_update or []):
            if getattr(u, "update_mode", None) not in ("sem-inc", "sem-add-imm"):
                continue
            lst = proc_events.setdefault(u.ant_name, [])
            prev = lst[-1][0] if lst else 0
            lst.append((prev + (u.update_value or 1), ins))

    import bisect

    def holds(sem, v):
        """Exact transitive closure of thresholds implied by sem >= v."""
        out = {}
        work = [(sem, v)]
        while work:
            s, t = work.pop()
            lst = proc_events.get(s)
            if not lst:
                continue
            ticks = [tk for tk, _ in lst]
            i = bisect.bisect_left(ticks, t)
            if i >= len(lst):
                continue
            implied = {}
            for j in range(i + 1):
                _, ins = lst[j]
                st = stream_of.get(id(ins))
                if st is not None:
                    for (s2, v2) in stream_cum[st[0]][st[1]].items():
                        if implied.get(s2, 0) < v2:
                            implied[s2] = v2
                else:
                    for (s2, v2) in waits_of.get(id(ins), []):
                        if implied.get(s2, 0) < v2:
                            implied[s2] = v2
            for s2, v2 in implied.items():
                if out.get(s2, 0) < v2:
                    out[s2] = v2
                    work.append((s2, v2))
        return out

    own_tick = {}
    for sem, lst in proc_events.items():
        for tick, ins in lst:
            own_tick[(id(ins), sem)] = tick

    pruned = 0
    for ins in insts:
        si = getattr(ins, "sync_info", None)
        if si is None or not si.on_wait or len(si.on_wait) < 2:
            continue
        ow = list(si.on_wait)
        kept = list(ow)
        for w in ow:
            if len(kept) == 1:
                break
            mine = own_tick.get((id(ins), w.ant_name))
            if mine is not None and w.wait_value <= mine - 1:
                kept.remove(w)
                pruned += 1
                continue
            others = [o for o in kept if o is not w]
            for o in others:
                h = holds(o.ant_name, o.wait_value)
                if h.get(w.ant_name, 0) >= w.wait_value:
                    kept.remove(w)
                    pruned += 1
                    break
        si.on_wait = kept
    return pruned


def _host_prep(x, conv_w, centroids):
    from concourse import mybir
    bf16np = mybir.dt.np(mybir.dt.bfloat16)
    fp8np = mybir.dt.np(mybir.dt.float8e4)

    x = np.ascontiguousarray(x, dtype=np.float32)
    L = H * W
    norm = np.sqrt((x.astype(np.float64) ** 2).sum(0))
    inv_norm = (1.0 / np.maximum(norm, 1e-12)).astype(np.float32)  # [H,W]
    xn = x * inv_norm[None]                                  # normalized x
    ii = np.arange(H, dtype=np.float32)
    mi = np.minimum(H - 1 - ii, ii)
    m = np.minimum(mi[:, None], mi[None, :]).astype(np.float32)
    m2 = m * m
    mask4 = m2 * m2                                          # [H,W]

    xpad8 = np.zeros((C, H + 2, W), fp8np)
    xpad8[:, 1:H + 1, :] = xn.astype(fp8np)
    xnb_pad = np.zeros(((H + 2) * W, C), bf16np)             # [Lpad, C] bf16
    xnb_pad[W:(H + 1) * W] = xn.reshape(C, L).T.astype(bf16np)
    sc_pad = np.zeros((H + 2) * W, np.float32)
    sc_pad[W:(H + 1) * W] = mask4.reshape(L)

    cwb = np.ascontiguousarray(
        conv_w.astype(np.float32).T.reshape(CT, 128, K)
        .transpose(1, 0, 2).reshape(128, CT * K)).astype(fp8np)

    # 5 banded 0/1 shift matrices: A_d[p', p] = 1 iff 128d + p' - p in D
    D = {di * W + dj for di in (-1, 0, 1) for dj in (-1, 0, 1)}
    shb = np.zeros((128, 5 * 128), bf16np)
    pp_, p_ = np.meshgrid(np.arange(128), np.arange(128), indexing="ij")
    for j, d in enumerate(range(-2, 3)):
        band = np.isin(128 * d + pp_ - p_, list(D))
        shb[:, j * 128:(j + 1) * 128] = band.astype(bf16np)

    V0, V1 = CNW[0][0], CNW[-1][1]
    NV = V1 - V0

    in_maps = []
    for core in range(M):
        r0 = core * RPC
        sl = slice(r0 * W, (r0 + RPC + 2) * W)               # slab, padded coords
        sc0c = sc_pad[sl].copy()
        sc0c[0:W] = 0.0                                      # halo rows give 0
        sc0c[(RPC + 1) * W:] = 0.0
        # xtb: [128, (t, ct, li)] = xn[ct*128+p, t*128+li] (fp8)
        xs = xpad8[:, r0:r0 + RPC + 2, :].reshape(C, Ls)
        xtb = np.ascontiguousarray(
            xs.reshape(CT, 128, NT, 128).transpose(1, 2, 0, 3)
            .reshape(128, NT * CT * 128))
        # xvb: [128, (t-V0, c)] = xnT[t*128+p, c], ones column; tiles V0..V1
        lo = r0 * W + V0 * 128                               # padded coords
        xv = np.ones((NV * 128, C1), bf16np)
        xv[:, 0:C] = xnb_pad[lo:lo + NV * 128]
        xvb = np.ascontiguousarray(
            xv.reshape(NV, 128, C1).transpose(1, 0, 2).reshape(128, NV * C1))
        in_maps.append({
            "xtb": xtb,
            "xvb": xvb,
            "cwb": cwb,
            "shb": shb,
            "cst": np.ascontiguousarray(sc0c.reshape(NT, 128).T,
                                        dtype=np.float32),
        })
    return in_maps


def _ensure_ntff_hook():
    """Install the axon NTFF profile hook if the image's antenv lacks it."""
    import types
    try:
        from antenv.axon_hooks import get_axon_ntff_profile_hook  # noqa: F401
        return
    except ImportError:
        pass
    if "/root/.axon_site" not in sys.path:
        sys.path.insert(0, "/root/.axon_site")
    from trn_agent_boot.trn_boot import _ntff_profile_via_ctypes
    hook = _ntff_profile_via_ctypes("/opt/axon/libaxon_pjrt.so")
    mod = types.ModuleType("antenv.axon_hooks")
    mod.get_axon_ntff_profile_hook = lambda: hook
    mod.set_axon_ntff_profile_hook = lambda h: None
    import antenv
    antenv.axon_hooks = mod
    sys.modules["antenv.axon_hooks"] = mod


def _install_neff_cache():
    """Cache compiled NEFFs across processes, keyed by BIR content hash."""
    import hashlib
    import shutil
    import concourse.bass2jax as b2j

    orig = b2j.compile_bir_kernel
    if getattr(orig, "_neff_cached", False):
        return

    def cached(bir_json, tmpdir, neff_name="file.neff"):
        h = hashlib.sha256(
            bir_json if isinstance(bir_json, bytes) else bir_json.encode()
        ).hexdigest()[:24]
        cdir = "/tmp/neff_cache"
        os.makedirs(cdir, exist_ok=True)
        cpath = os.path.join(cdir, h + ".neff")
        if os.path.exists(cpath):
            dst = os.path.join(tmpdir, neff_name)
            os.makedirs(tmpdir, exist_ok=True)
            shutil.copy(cpath, dst)
            return dst
        out = orig(bir_json, tmpdir, neff_name=neff_name)
        shutil.copy(out, cpath)
        return out

    cached._neff_cached = True
    b2j.compile_bir_kernel = cached


def kernel(x, conv_w, centroids):
    import concourse.bass_utils as bu
    from concourse.bass_utils import run_bass_kernel_spmd
    _install_neff_cache()
    if TRACE:
        _ensure_ntff_hook()
        bu.upload_artifacts = lambda tmpdir: "local://" + tmpdir

    if "nc" not in _CACHE:
        _CACHE["nc"] = _build_nc()
    nc = _CACHE["nc"]
    in_maps = _host_prep(np.asarray(x), np.asarray(conv_w), np.asarray(centroids))
    res = run_bass_kernel_spmd(nc, in_maps, list(range(M)), trace=TRACE)
    _CACHE["last"] = res
    red = np.zeros((K, C1), np.float32)
    for r in res.results:
        red += np.asarray(r["y"], dtype=np.float32)
    vlad = red[:, :C] - red[:, C:C1] * np.asarray(centroids, np.float32)
    vlad /= np.maximum(np.sqrt((vlad ** 2).sum(1))[:, None], 1e-12)
    v = vlad.reshape(1, K * C)
    v /= np.maximum(np.sqrt((v ** 2).sum()), 1e-12)
    return v.astype(np.float32)
